# revision 19
# baseline (speedup 1.0000x reference)
"""DigitCaps dynamic-routing kernel for 8x TRN2 NeuronCores.

Strategy: shard in_caps (2048 -> 256/core). Each core computes its u_hat
slice with PE (block-diag stationary x trick), stores it bf16 in DRAM,
then runs 4 routing iterations streaming u_hat. Per-iteration partial
s = sum_i c*u_hat is reduced on PE (ones-matmul over the i partition
axis) and all-reduced across the 8 cores; the tiny squash/V update is
replicated on every core.

Math notes:
  b_r = u_hat * V_r elementwise with V_r = sum_{t<r} v_t (b_0 = 0), so the
  routing logits never need to be materialized across iterations.
  softmax uses a constant shift T0 (exact: constant along the softmax axis).
"""

import numpy as np
import ml_dtypes

B, I_TOT, KD, O, D = 32, 2048, 16, 64, 32
OD = O * D
NCORES = 8
IC = I_TOT // NCORES       # 256 in_caps per core
G = IC // 8                # 32 groups of 8 i
ITERS = 5
EPS = 1e-8
T0 = 20.0
BF = ml_dtypes.bfloat16

_CACHE = {}


def _prep_core(x, W, c):
    """Host-side per-core input prep (slicing + relayout + bf16 cast)."""
    i0 = c * IC
    xs = np.ascontiguousarray(x[:, i0:i0 + IC, :], np.float32)   # [B, 256, 16]
    Ws = W[i0:i0 + IC]                                           # [256, O, D, KD]

    # W_st[g]: [128 rows=(il8,k16), OD cols=(o,d)]
    w_st = np.ascontiguousarray(
        Ws.reshape(G, 8, O, D, KD).transpose(0, 1, 4, 2, 3).reshape(G, 128, OD)
    ).astype(BF)

    # x_blk: [128 rows=(h2,il4,k16), G, 128 cols=(il4',b)], block-diag in il4
    xr = xs.reshape(B, G, 2, 4, KD)                              # b g h il k
    xb = np.zeros((2, 4, KD, G, 4, B), np.float32)
    for h in range(2):
        for il in range(4):
            xb[h, il, :, :, il, :] = xr[:, :, h, il, :].transpose(2, 1, 0)
    x_blk = np.ascontiguousarray(xb.reshape(128, G, 128)).astype(BF)

    # xT: [128 rows=(il8,k16), G*B] for the s0 full reduction
    xT = np.ascontiguousarray(
        xs.reshape(B, G, 8, KD).transpose(1, 2, 3, 0).reshape(G, 128, B)
        .transpose(1, 0, 2).reshape(128, G * B)
    ).astype(BF)

    return {"w_st": w_st, "x_blk": x_blk, "xT": xT}


def _squash(nc, sp, dt, AF, s_dram, r, V_f, V_bf, V_dram, v_out, ebias):
    """s [B, OD] f32 in DRAM -> v; update V (or write final output)."""
    s_sb = sp.tile([B, OD], dt.float32, tag="sq_s", name="sq_s")
    nc.sync.dma_start(s_sb[:], s_dram[:])
    sq = sp.tile([B, OD], dt.float32, tag="sq_sq", name="sq_sq")
    nc.vector.tensor_mul(sq[:], s_sb[:], s_sb[:])
    n2 = sp.tile([B, O], dt.float32, tag="sq_n2", name="sq_n2")
    import concourse.mybir as mybir
    nc.vector.tensor_reduce(
        n2[:], sq[:].rearrange("p (o d) -> p o d", d=D),
        axis=mybir.AxisListType.X, op=mybir.AluOpType.add)
    # 1/sqrt(n2+eps) via exp(-0.5*ln(n2+eps)) (exp/ln share one ACT table set)
    ln = sp.tile([B, O], dt.float32, tag="sq_ln", name="sq_ln")
    nc.scalar.activation(ln[:], n2[:], AF.Ln, bias=ebias[0:B, :])
    rsq = sp.tile([B, O], dt.float32, tag="sq_rsq", name="sq_rsq")
    nc.scalar.activation(rsq[:], ln[:], AF.Exp, scale=-0.5)
    np1 = sp.tile([B, O], dt.float32, tag="sq_np1", name="sq_np1")
    nc.vector.tensor_scalar_add(np1[:], n2[:], 1.0)
    inv = sp.tile([B, O], dt.float32, tag="sq_inv", name="sq_inv")
    nc.vector.reciprocal(inv[:], np1[:])
    f1 = sp.tile([B, O], dt.float32, tag="sq_f1", name="sq_f1")
    nc.vector.tensor_mul(f1[:], n2[:], rsq[:])
    f2 = sp.tile([B, O], dt.float32, tag="sq_f2", name="sq_f2")
    nc.vector.tensor_mul(f2[:], f1[:], inv[:])
    vn = sp.tile([B, OD], dt.float32, tag="sq_vn", name="sq_vn")
    nc.vector.tensor_mul(
        vn[:].rearrange("p (o d) -> p o d", d=D),
        s_sb[:].rearrange("p (o d) -> p o d", d=D),
        f2[:].unsqueeze(2).broadcast_to([B, O, D]))
    if v_out is not None:
        nc.sync.dma_start(v_out.rearrange("b o d -> b (o d)"), vn[:])
    else:
        if r == 0:
            nc.vector.tensor_copy(V_f[:], vn[:])
        else:
            nc.vector.tensor_add(V_f[:], V_f[:], vn[:])
        nc.vector.tensor_copy(V_bf[:], V_f[:])
        nc.sync.dma_start(V_dram[:], V_bf[:])


def _build():
    import concourse.bass as bass  # noqa: F401
    import concourse.bacc as bacc
    import concourse.mybir as mybir
    import concourse.tile as tile

    dt = mybir.dt
    AF = mybir.ActivationFunctionType
    ALU = mybir.AluOpType

    nc = bacc.Bacc("TRN2", target_bir_lowering=False, debug=False,
                   num_devices=NCORES)
    w_st = nc.dram_tensor("w_st", [G, 128, OD], dt.bfloat16,
                          kind="ExternalInput").ap()
    x_blk = nc.dram_tensor("x_blk", [128, G, 128], dt.bfloat16,
                           kind="ExternalInput").ap()
    xT = nc.dram_tensor("xT", [128, G * B], dt.bfloat16,
                        kind="ExternalInput").ap()
    v_out = nc.dram_tensor("v_out", [B, O, D], dt.float32,
                           kind="ExternalOutput").ap()
    dbg_u = nc.dram_tensor("dbg_u", [128, OD], dt.bfloat16,
                           kind="ExternalOutput").ap()
    dbg_sp = nc.dram_tensor("dbg_sp", [B, OD], dt.float32,
                            kind="ExternalOutput").ap()
    dbg_sf = nc.dram_tensor("dbg_sf", [B, OD], dt.float32,
                            kind="ExternalOutput").ap()
    dbg_V = nc.dram_tensor("dbg_V", [B, OD], dt.float32,
                           kind="ExternalOutput").ap()
    dbg_s4 = nc.dram_tensor("dbg_s4", [B, OD], dt.float32,
                            kind="ExternalOutput").ap()
    dbg_sr = nc.dram_tensor("dbg_sr", [ITERS - 1, B, OD], dt.float32,
                            kind="ExternalOutput").ap()
    dbg_spr = nc.dram_tensor("dbg_spr", [ITERS - 1, B, OD], dt.float32,
                             kind="ExternalOutput").ap()
    dbg_Vr = nc.dram_tensor("dbg_Vr", [ITERS - 1, B, OD], dt.bfloat16,
                            kind="ExternalOutput").ap()
    dbg_vb = nc.dram_tensor("dbg_vb", [128, OD], dt.bfloat16,
                            kind="ExternalOutput").ap()

    groups = [list(range(NCORES))]

    with tile.TileContext(nc) as tc:
        with tc.tile_pool(name="dram", bufs=1, space="DRAM") as dp, \
             tc.tile_pool(name="smalls", bufs=1) as sp:
            u_dram = dp.tile([B, IC, OD], dt.bfloat16, tag="u_dram", name="u_dram")
            ar_in = [dp.tile([B, OD], dt.float32, tag=f"ar_in{r}", name=f"ar_in{r}")
                     for r in range(ITERS)]
            ar_out = [dp.tile([B, OD], dt.float32, tag=f"ar_out{r}", name=f"ar_out{r}",
                              addr_space="Shared")
                      for r in range(ITERS)]
            V_dram = dp.tile([B, OD], dt.bfloat16, tag="V_dram", name="V_dram")
            V_f = sp.tile([B, OD], dt.float32, tag="V_f", name="V_f")
            V_bf = sp.tile([B, OD], dt.bfloat16, tag="V_bf", name="V_bf")
            ones = sp.tile([128, 1], dt.bfloat16, tag="ones", name="ones")
            nc.vector.memset(ones[:], 1.0)
            tbias = sp.tile([128, 1], dt.float32, tag="tbias", name="tbias")
            nc.vector.memset(tbias[:], -T0)
            ebias = sp.tile([128, 1], dt.float32, tag="ebias", name="ebias")
            nc.vector.memset(ebias[:], EPS)

            # ---------------- phase A: u_hat + s0 ----------------
            with tc.tile_pool(name="wp", bufs=1) as wp:
                wt = []
                for g in range(G):
                    t = wp.tile([128, OD], dt.bfloat16, tag=f"w{g}", name=f"w{g}")
                    nc.sync.dma_start(t[:], w_st[g])
                    wt.append(t)
                xbt = wp.tile([128, G * 128], dt.bfloat16, tag="xbt", name="xbt")
                nc.sync.dma_start(xbt[:], x_blk.rearrange("p g m -> p (g m)"))
                xTt = wp.tile([128, G * B], dt.bfloat16, tag="xTt", name="xTt")
                nc.sync.dma_start(xTt[:], xT)

                with tc.tile_pool(name="paps", bufs=2, space="PSUM") as pap, \
                     tc.tile_pool(name="ucp", bufs=4) as ucp:
                    for g in range(G):
                        for h in range(2):
                            pu = pap.tile([128, OD], dt.float32, tag="pu",
                                          name="pu")
                            for n in range(4):
                                nc.tensor.matmul(
                                    pu[:, n * 512:(n + 1) * 512],
                                    lhsT=xbt[64 * h:64 * (h + 1),
                                             g * 128:(g + 1) * 128],
                                    rhs=wt[g][64 * h:64 * (h + 1),
                                              n * 512:(n + 1) * 512],
                                    start=True, stop=True)
                            ut = ucp.tile([128, OD], dt.bfloat16, tag="uc",
                                          name="uc")
                            if h == 0:
                                nc.vector.tensor_copy(ut[:], pu[:])
                            else:
                                nc.scalar.copy(ut[:], pu[:])
                            gs = g * 8 + h * 4
                            for il in range(4):
                                nc.sync.dma_start(
                                    u_dram[:, gs + il, :],
                                    ut[il * 32:(il + 1) * 32, :])

                with tc.tile_pool(name="s0ps", bufs=1, space="PSUM") as s0p:
                    ps0 = s0p.tile([B, OD], dt.float32, tag="ps0", name="ps0")
                    for g in range(G):
                        for n in range(4):
                            nc.tensor.matmul(
                                ps0[:, n * 512:(n + 1) * 512],
                                lhsT=xTt[:, g * B:(g + 1) * B],
                                rhs=wt[g][:, n * 512:(n + 1) * 512],
                                start=(g == 0), stop=(g == G - 1))
                    s_stage = sp.tile([B, OD], dt.float32, tag="s_stage", name="s_stage")
                    nc.scalar.mul(s_stage[:], ps0[:], 1.0 / O)
                    nc.sync.dma_start(ar_in[0][:], s_stage[:])

            nc.gpsimd.collective_compute(
                "AllReduce", ALU.add, groups,
                ins=[ar_in[0].opt()], outs=[ar_out[0].opt()])
            _squash(nc, sp, dt, AF, ar_out[0], 0, V_f, V_bf, V_dram, None,
                    ebias)
            nc.sync.dma_start(dbg_u, u_dram[0, 0:128, :])
            nc.sync.dma_start(dbg_sp, ar_in[0][:])
            nc.sync.dma_start(dbg_sf, ar_out[0][:])

            # ---------------- routing iterations ----------------
            with tc.tile_pool(name="up", bufs=4) as up, \
                 tc.tile_pool(name="vbp", bufs=2) as vbp, \
                 tc.tile_pool(name="tp", bufs=3) as tp, \
                 tc.tile_pool(name="ep", bufs=3) as ep, \
                 tc.tile_pool(name="cp", bufs=3) as cp, \
                 tc.tile_pool(name="pp", bufs=3) as pp, \
                 tc.tile_pool(name="zp", bufs=3) as zp, \
                 tc.tile_pool(name="zrp", bufs=4) as zrp, \
                 tc.tile_pool(name="sps", bufs=2, space="PSUM") as sps:
                for r in range(1, ITERS):
                    nc.sync.dma_start(dbg_Vr[r - 1], V_dram[:])
                    for b in range(B):
                        vbt = vbp.tile([128, OD], dt.bfloat16, tag="vb", name="vb")
                        nc.sync.dma_start(
                            vbt[:],
                            V_dram[b:b + 1, :].broadcast_to([128, OD]))
                        if r == 1 and b == 1:
                            nc.sync.dma_start(dbg_vb, vbt[:])
                        ps = sps.tile([1, OD], dt.float32, tag="ps", name="ps")
                        for it in range(2):
                            ut = up.tile([128, OD], dt.bfloat16, tag="u", name="u")
                            nc.sync.dma_start(
                                ut[:], u_dram[b, it * 128:(it + 1) * 128, :])
                            tt = tp.tile([128, OD], dt.bfloat16, tag="t", name="t")
                            nc.vector.tensor_mul(tt[:], ut[:], vbt[:])
                            et = ep.tile([128, OD], dt.bfloat16, tag="e", name="e")
                            nc.scalar.activation(et[:], tt[:], AF.Exp,
                                                 bias=tbias[:])
                            zt = zp.tile([128, 1024], dt.bfloat16, tag="z", name="z")
                            nc.vector.tensor_add(zt[:], et[:, 0:1024],
                                                 et[:, 1024:2048])
                            for w in (16, 8, 4, 2, 1):
                                nc.vector.tensor_add(
                                    zt[:, 0:w * 32], zt[:, 0:w * 32],
                                    zt[:, w * 32:2 * w * 32])
                            zf = zrp.tile([128, D], dt.float32, tag="zf", name="zf")
                            nc.vector.tensor_copy(zf[:], zt[:, 0:D])
                            zrf = zrp.tile([128, D], dt.float32, tag="zrf", name="zrf")
                            nc.vector.reciprocal_approx_fast(zrf[:], zf[:])
                            zrb = zrp.tile([128, D], dt.bfloat16, tag="zrb", name="zrb")
                            nc.vector.tensor_copy(zrb[:], zrf[:])
                            ct = cp.tile([128, OD], dt.bfloat16, tag="c", name="c")
                            nc.vector.tensor_mul(
                                ct[:].rearrange("p (o d) -> p o d", d=D),
                                et[:].rearrange("p (o d) -> p o d", d=D),
                                zrb[:].unsqueeze(1).broadcast_to([128, O, D]))
                            pt = pp.tile([128, OD], dt.bfloat16, tag="pt", name="pt")
                            nc.vector.tensor_mul(pt[:], ct[:], ut[:])
                            for n in range(4):
                                nc.tensor.matmul(
                                    ps[:, n * 512:(n + 1) * 512],
                                    lhsT=ones[:],
                                    rhs=pt[:, n * 512:(n + 1) * 512],
                                    start=(it == 0), stop=(it == 1))
                        sc = sp.tile([1, OD], dt.float32, tag="s_row", name="s_row")
                        nc.scalar.copy(sc[:], ps[:])
                        nc.sync.dma_start(ar_in[r][b:b + 1, :], sc[:])
                    nc.sync.dma_start(dbg_spr[r - 1], ar_in[r][:])
                    nc.gpsimd.collective_compute(
                        "AllReduce", ALU.add, groups,
                        ins=[ar_in[r].opt()], outs=[ar_out[r].opt()])
                    nc.sync.dma_start(dbg_sr[r - 1], ar_out[r][:])
                    _squash(nc, sp, dt, AF, ar_out[r], r, V_f, V_bf, V_dram,
                            v_out if r == ITERS - 1 else None, ebias)
            nc.sync.dma_start(dbg_V, V_f[:])
            nc.sync.dma_start(dbg_s4, ar_out[ITERS - 1][:])

    nc.compile()
    return nc


def _get_prog():
    if "nc" not in _CACHE:
        _CACHE["nc"] = _build()
    return _CACHE["nc"]


def kernel(x, W):
    x = np.asarray(x, np.float32)
    W = np.asarray(W, np.float32)
    nc = _get_prog()
    in_maps = [_prep_core(x, W, c) for c in range(NCORES)]
    from concourse import bass_utils
    res = bass_utils.run_bass_kernel_spmd(
        nc, in_maps, core_ids=list(range(NCORES)))
    return np.ascontiguousarray(res.results[0]["v_out"].astype(np.float32))


# revision 23
# speedup vs baseline: 1.3056x; 1.3056x over previous
"""DigitCaps dynamic-routing kernel for 8x TRN2 NeuronCores.

Strategy: shard in_caps (2048 -> 256/core). Each core computes its u_hat
slice with PE (block-diag stationary x trick), stores it bf16 in DRAM,
then runs 4 routing iterations streaming u_hat. Per-iteration partial
s = sum_i c*u_hat is reduced on PE (ones-matmul over the i partition
axis) and all-reduced across the 8 cores; the tiny squash/V update is
replicated on every core.

Math notes:
  b_r = u_hat * V_r elementwise with V_r = sum_{t<r} v_t (b_0 = 0), so the
  routing logits never need to be materialized across iterations.
  softmax uses a constant shift T0 (exact: constant along the softmax axis).
"""

import os
import numpy as np
import ml_dtypes

DC_NO_COLLECTIVE = os.environ.get("DC_NO_COLLECTIVE", "0") == "1"
DC_PHASE_A_ONLY = os.environ.get("DC_PHASE_A_ONLY", "0") == "1"

B, I_TOT, KD, O, D = 32, 2048, 16, 64, 32
OD = O * D
NCORES = 8
IC = I_TOT // NCORES       # 256 in_caps per core
G = IC // 8                # 32 groups of 8 i
ITERS = 5
EPS = 1e-8
T0 = 20.0
BF = ml_dtypes.bfloat16

_CACHE = {}


def _prep_core(x, W, c):
    """Host-side per-core input prep (slicing + relayout + bf16 cast)."""
    i0 = c * IC
    xs = np.ascontiguousarray(x[:, i0:i0 + IC, :], np.float32)   # [B, 256, 16]
    Ws = W[i0:i0 + IC]                                           # [256, O, D, KD]

    # W_st[g]: [128 rows=(il8,k16), OD cols=(o,d)]
    w_st = np.ascontiguousarray(
        Ws.reshape(G, 8, O, D, KD).transpose(0, 1, 4, 2, 3).reshape(G, 128, OD)
    ).astype(BF)

    # x_blk: [128 rows=(h2,il4,k16), G, 128 cols=(il4',b)], block-diag in il4
    xr = xs.reshape(B, G, 2, 4, KD)                              # b g h il k
    xb = np.zeros((2, 4, KD, G, 4, B), np.float32)
    for h in range(2):
        for il in range(4):
            xb[h, il, :, :, il, :] = xr[:, :, h, il, :].transpose(2, 1, 0)
    x_blk = np.ascontiguousarray(xb.reshape(128, G, 128)).astype(BF)

    # xT: [128 rows=(il8,k16), G*B] for the s0 full reduction
    xT = np.ascontiguousarray(
        xs.reshape(B, G, 8, KD).transpose(1, 2, 3, 0).reshape(G, 128, B)
        .transpose(1, 0, 2).reshape(128, G * B)
    ).astype(BF)

    return {"w_st": w_st, "x_blk": x_blk, "xT": xT}


def _squash(nc, sp, dt, AF, s_dram, r, V_f, V_bf, V_dram, v_out, ebias):
    """s [B, OD] f32 in DRAM -> v; update V (or write final output)."""
    s_sb = sp.tile([B, OD], dt.float32, tag="sq_s", name="sq_s")
    nc.sync.dma_start(s_sb[:], s_dram[:])
    sq = sp.tile([B, OD], dt.float32, tag="sq_sq", name="sq_sq")
    nc.vector.tensor_mul(sq[:], s_sb[:], s_sb[:])
    n2 = sp.tile([B, O], dt.float32, tag="sq_n2", name="sq_n2")
    import concourse.mybir as mybir
    nc.vector.tensor_reduce(
        n2[:], sq[:].rearrange("p (o d) -> p o d", d=D),
        axis=mybir.AxisListType.X, op=mybir.AluOpType.add)
    # 1/sqrt(n2+eps) via exp(-0.5*ln(n2+eps)) (exp/ln share one ACT table set)
    ln = sp.tile([B, O], dt.float32, tag="sq_ln", name="sq_ln")
    nc.scalar.activation(ln[:], n2[:], AF.Ln, bias=ebias[0:B, :])
    rsq = sp.tile([B, O], dt.float32, tag="sq_rsq", name="sq_rsq")
    nc.scalar.activation(rsq[:], ln[:], AF.Exp, scale=-0.5)
    np1 = sp.tile([B, O], dt.float32, tag="sq_np1", name="sq_np1")
    nc.vector.tensor_scalar_add(np1[:], n2[:], 1.0)
    inv = sp.tile([B, O], dt.float32, tag="sq_inv", name="sq_inv")
    nc.vector.reciprocal(inv[:], np1[:])
    f1 = sp.tile([B, O], dt.float32, tag="sq_f1", name="sq_f1")
    nc.vector.tensor_mul(f1[:], n2[:], rsq[:])
    f2 = sp.tile([B, O], dt.float32, tag="sq_f2", name="sq_f2")
    nc.vector.tensor_mul(f2[:], f1[:], inv[:])
    vn = sp.tile([B, OD], dt.float32, tag="sq_vn", name="sq_vn")
    nc.vector.tensor_mul(
        vn[:].rearrange("p (o d) -> p o d", d=D),
        s_sb[:].rearrange("p (o d) -> p o d", d=D),
        f2[:].unsqueeze(2).broadcast_to([B, O, D]))
    if v_out is not None:
        nc.sync.dma_start(v_out.rearrange("b o d -> b (o d)"), vn[:])
    else:
        if r == 0:
            nc.vector.tensor_copy(V_f[:], vn[:])
        else:
            nc.vector.tensor_add(V_f[:], V_f[:], vn[:])
        nc.vector.tensor_copy(V_bf[:], V_f[:])
        nc.sync.dma_start(V_dram[:], V_bf[:])


def _build():
    import concourse.bass as bass  # noqa: F401
    import concourse.bacc as bacc
    import concourse.mybir as mybir
    import concourse.tile as tile

    dt = mybir.dt
    AF = mybir.ActivationFunctionType
    ALU = mybir.AluOpType

    nc = bacc.Bacc("TRN2", target_bir_lowering=False, debug=False,
                   num_devices=NCORES)
    w_st = nc.dram_tensor("w_st", [G, 128, OD], dt.bfloat16,
                          kind="ExternalInput").ap()
    x_blk = nc.dram_tensor("x_blk", [128, G, 128], dt.bfloat16,
                           kind="ExternalInput").ap()
    xT = nc.dram_tensor("xT", [128, G * B], dt.bfloat16,
                        kind="ExternalInput").ap()
    v_out = nc.dram_tensor("v_out", [B, O, D], dt.float32,
                           kind="ExternalOutput").ap()
    dbg_u = nc.dram_tensor("dbg_u", [128, OD], dt.bfloat16,
                           kind="ExternalOutput").ap()
    dbg_sp = nc.dram_tensor("dbg_sp", [B, OD], dt.float32,
                            kind="ExternalOutput").ap()
    dbg_sf = nc.dram_tensor("dbg_sf", [B, OD], dt.float32,
                            kind="ExternalOutput").ap()
    dbg_V = nc.dram_tensor("dbg_V", [B, OD], dt.float32,
                           kind="ExternalOutput").ap()
    dbg_s4 = nc.dram_tensor("dbg_s4", [B, OD], dt.float32,
                            kind="ExternalOutput").ap()
    dbg_sr = nc.dram_tensor("dbg_sr", [ITERS - 1, B, OD], dt.float32,
                            kind="ExternalOutput").ap()
    dbg_spr = nc.dram_tensor("dbg_spr", [ITERS - 1, B, OD], dt.float32,
                             kind="ExternalOutput").ap()
    dbg_Vr = nc.dram_tensor("dbg_Vr", [ITERS - 1, B, OD], dt.bfloat16,
                            kind="ExternalOutput").ap()
    dbg_vb = nc.dram_tensor("dbg_vb", [128, OD], dt.bfloat16,
                            kind="ExternalOutput").ap()

    groups = [list(range(NCORES))]

    with tile.TileContext(nc) as tc:
        with tc.tile_pool(name="dram", bufs=1, space="DRAM") as dp, \
             tc.tile_pool(name="smalls", bufs=1) as sp:
            u_dram = dp.tile([B, IC, OD], dt.bfloat16, tag="u_dram", name="u_dram")
            ar_in = [dp.tile([B, OD], dt.float32, tag=f"ar_in{r}", name=f"ar_in{r}")
                     for r in range(ITERS)]
            ar_out = [dp.tile([B, OD], dt.float32, tag=f"ar_out{r}", name=f"ar_out{r}",
                              addr_space="Shared")
                      for r in range(ITERS)]
            V_dram = dp.tile([B, OD], dt.bfloat16, tag="V_dram", name="V_dram")
            V_f = sp.tile([B, OD], dt.float32, tag="V_f", name="V_f")
            V_bf = sp.tile([B, OD], dt.bfloat16, tag="V_bf", name="V_bf")
            ones = sp.tile([128, 1], dt.bfloat16, tag="ones", name="ones")
            nc.vector.memset(ones[:], 1.0)
            tbias = sp.tile([128, 1], dt.float32, tag="tbias", name="tbias")
            nc.vector.memset(tbias[:], -T0)
            ebias = sp.tile([128, 1], dt.float32, tag="ebias", name="ebias")
            nc.vector.memset(ebias[:], EPS)

            # ---------------- phase A: u_hat + s0 ----------------
            with tc.tile_pool(name="wp", bufs=1) as wp:
                wt = []
                for g in range(G):
                    t = wp.tile([128, OD], dt.bfloat16, tag=f"w{g}", name=f"w{g}")
                    nc.sync.dma_start(t[:], w_st[g])
                    wt.append(t)
                xbt = wp.tile([128, G * 128], dt.bfloat16, tag="xbt", name="xbt")
                nc.sync.dma_start(xbt[:], x_blk.rearrange("p g m -> p (g m)"))
                xTt = wp.tile([128, G * B], dt.bfloat16, tag="xTt", name="xTt")
                nc.sync.dma_start(xTt[:], xT)

                with tc.tile_pool(name="paps", bufs=2, space="PSUM") as pap, \
                     tc.tile_pool(name="ucp", bufs=4) as ucp:
                    for g in range(G):
                        for h in range(2):
                            pu = pap.tile([128, OD], dt.float32, tag="pu",
                                          name="pu")
                            for n in range(4):
                                nc.tensor.matmul(
                                    pu[:, n * 512:(n + 1) * 512],
                                    lhsT=xbt[64 * h:64 * (h + 1),
                                             g * 128:(g + 1) * 128],
                                    rhs=wt[g][64 * h:64 * (h + 1),
                                              n * 512:(n + 1) * 512],
                                    start=True, stop=True)
                            ut = ucp.tile([128, OD], dt.bfloat16, tag="uc",
                                          name="uc")
                            if h == 0:
                                nc.vector.tensor_copy(ut[:], pu[:])
                            else:
                                nc.scalar.copy(ut[:], pu[:])
                            gs = g * 8 + h * 4
                            for il in range(4):
                                nc.sync.dma_start(
                                    u_dram[:, gs + il, :],
                                    ut[il * 32:(il + 1) * 32, :])

                with tc.tile_pool(name="s0ps", bufs=1, space="PSUM") as s0p:
                    ps0 = s0p.tile([B, OD], dt.float32, tag="ps0", name="ps0")
                    for g in range(G):
                        for n in range(4):
                            nc.tensor.matmul(
                                ps0[:, n * 512:(n + 1) * 512],
                                lhsT=xTt[:, g * B:(g + 1) * B],
                                rhs=wt[g][:, n * 512:(n + 1) * 512],
                                start=(g == 0), stop=(g == G - 1))
                    s_stage = sp.tile([B, OD], dt.float32, tag="s_stage", name="s_stage")
                    nc.scalar.mul(s_stage[:], ps0[:], 1.0 / O)
                    nc.sync.dma_start(ar_in[0][:], s_stage[:])

            def allreduce(r):
                if DC_NO_COLLECTIVE:
                    nc.sync.dma_start(ar_out[r][:], ar_in[r][:])
                else:
                    nc.gpsimd.collective_compute(
                        "AllReduce", ALU.add, groups,
                        ins=[ar_in[r].opt()], outs=[ar_out[r].opt()])

            allreduce(0)
            _squash(nc, sp, dt, AF, ar_out[0], 0, V_f, V_bf, V_dram, None,
                    ebias)
            nc.sync.dma_start(dbg_u, u_dram[0, 0:128, :])
            nc.sync.dma_start(dbg_sp, ar_in[0][:])
            nc.sync.dma_start(dbg_sf, ar_out[0][:])

            # ---------------- routing iterations ----------------
            with tc.tile_pool(name="up", bufs=4) as up, \
                 tc.tile_pool(name="vbp", bufs=2) as vbp, \
                 tc.tile_pool(name="tp", bufs=3) as tp, \
                 tc.tile_pool(name="ep", bufs=3) as ep, \
                 tc.tile_pool(name="cp", bufs=3) as cp, \
                 tc.tile_pool(name="pp", bufs=3) as pp, \
                 tc.tile_pool(name="zp", bufs=3) as zp, \
                 tc.tile_pool(name="zrp", bufs=4) as zrp, \
                 tc.tile_pool(name="sps", bufs=2, space="PSUM") as sps:
                for r in range(1, 1 if DC_PHASE_A_ONLY else ITERS):
                    nc.sync.dma_start(dbg_Vr[r - 1], V_dram[:])
                    for b in range(B):
                        vbt = vbp.tile([128, OD], dt.bfloat16, tag="vb", name="vb")
                        nc.sync.dma_start(
                            vbt[:],
                            V_dram[b:b + 1, :].broadcast_to([128, OD]))
                        if r == 1 and b == 1:
                            nc.sync.dma_start(dbg_vb, vbt[:])
                        ps = sps.tile([1, OD], dt.float32, tag="ps", name="ps")
                        for it in range(2):
                            ut = up.tile([128, OD], dt.bfloat16, tag="u", name="u")
                            nc.sync.dma_start(
                                ut[:], u_dram[b, it * 128:(it + 1) * 128, :])
                            tt = tp.tile([128, OD], dt.bfloat16, tag="t", name="t")
                            nc.vector.tensor_mul(tt[:], ut[:], vbt[:])
                            et = ep.tile([128, OD], dt.bfloat16, tag="e", name="e")
                            nc.scalar.activation(et[:], tt[:], AF.Exp,
                                                 bias=tbias[:])
                            zt = zp.tile([128, 1024], dt.bfloat16, tag="z", name="z")
                            nc.vector.tensor_add(zt[:], et[:, 0:1024],
                                                 et[:, 1024:2048])
                            for w in (16, 8, 4, 2, 1):
                                nc.vector.tensor_add(
                                    zt[:, 0:w * 32], zt[:, 0:w * 32],
                                    zt[:, w * 32:2 * w * 32])
                            zf = zrp.tile([128, D], dt.float32, tag="zf", name="zf")
                            nc.vector.tensor_copy(zf[:], zt[:, 0:D])
                            zrf = zrp.tile([128, D], dt.float32, tag="zrf", name="zrf")
                            nc.vector.reciprocal_approx_fast(zrf[:], zf[:])
                            zrb = zrp.tile([128, D], dt.bfloat16, tag="zrb", name="zrb")
                            nc.vector.tensor_copy(zrb[:], zrf[:])
                            ct = cp.tile([128, OD], dt.bfloat16, tag="c", name="c")
                            nc.vector.tensor_mul(
                                ct[:].rearrange("p (o d) -> p o d", d=D),
                                et[:].rearrange("p (o d) -> p o d", d=D),
                                zrb[:].unsqueeze(1).broadcast_to([128, O, D]))
                            pt = pp.tile([128, OD], dt.bfloat16, tag="pt", name="pt")
                            nc.vector.tensor_mul(pt[:], ct[:], ut[:])
                            for n in range(4):
                                nc.tensor.matmul(
                                    ps[:, n * 512:(n + 1) * 512],
                                    lhsT=ones[:],
                                    rhs=pt[:, n * 512:(n + 1) * 512],
                                    start=(it == 0), stop=(it == 1))
                        sc = sp.tile([1, OD], dt.float32, tag="s_row", name="s_row")
                        nc.scalar.copy(sc[:], ps[:])
                        nc.sync.dma_start(ar_in[r][b:b + 1, :], sc[:])
                    nc.sync.dma_start(dbg_spr[r - 1], ar_in[r][:])
                    allreduce(r)
                    nc.sync.dma_start(dbg_sr[r - 1], ar_out[r][:])
                    _squash(nc, sp, dt, AF, ar_out[r], r, V_f, V_bf, V_dram,
                            v_out if r == ITERS - 1 else None, ebias)
            nc.sync.dma_start(dbg_V, V_f[:])
            nc.sync.dma_start(dbg_s4, ar_out[ITERS - 1][:])

    nc.compile()
    return nc


def _get_prog():
    if "nc" not in _CACHE:
        _CACHE["nc"] = _build()
    return _CACHE["nc"]


def kernel(x, W):
    x = np.asarray(x, np.float32)
    W = np.asarray(W, np.float32)
    nc = _get_prog()
    in_maps = [_prep_core(x, W, c) for c in range(NCORES)]
    from concourse import bass_utils
    res = bass_utils.run_bass_kernel_spmd(
        nc, in_maps, core_ids=list(range(NCORES)))
    return np.ascontiguousarray(res.results[0]["v_out"].astype(np.float32))
